# revision 1
# baseline (speedup 1.0000x reference)
"""Trainium2 kernel for DetContrastiveLoss (embedding_lookup).

Reference semantics (buggy original preserved): only the LAST batch element of
spatial_features_2d / gt_boxes is used.  500 box centers are mapped to pixel
indices, the 128-channel feature vector at each pixel is gathered, L2-normalized,
and a 500x500 cosine-similarity contrastive loss (log_softmax + label pick) is
reduced to a scalar.

Device strategy (8 NeuronCores, no collectives):
  Pass 1 (gather, channel-sharded): core k holds channels [16k, 16k+16) of the
     last batch element's feature map (16 MB table shard).  A single indirect
     DMA gathers its 16x500 feature slice: idx[c, n] = c*H*W + pix[n] indexes a
     flat view of the shard, landing the result directly in [C_shard, N] layout
     (channels on partitions) -- exactly the matmul operand layout.
  Host: concatenates the 8 slices into feats [128, 500] (256 KB).
  Pass 2 (loss, row-sharded): every core gets feats [128, 500] plus its own
     column block lhs = feats[:, rows_k] (63 rows/core).  Each core normalizes
     columns (via an all-ones [128,128] matmul that yields the column
     sum-of-squares already broadcast over partitions), computes its [63, 500]
     block of sim = (f @ f.T)/T, log_softmax over the free axis, picks the
     labeled entry per row, and returns a masked partial sum.  Host adds the 8
     partial scalars and applies -LOSS_SCALE/N.
"""

import contextlib
import ctypes
import os
import sys
import types

import numpy as np

from concourse import bass, mybir
from concourse import bass_utils as _bass_utils
from concourse.bass_utils import run_bass_kernel_spmd
from concourse.tile import TileContext

# Problem geometry (hardcoded per spec nn_DetContrastiveLoss_72636486910298).
B, C, H, W = 4, 128, 512, 512
HW = H * W
N = 500
NCORES = 8
CPC = C // NCORES            # channels per core in the gather pass: 16
# rows per core in the loss pass; 64 keeps the G-matmul's output partition
# count (M) a power of two — M=63 produced garbage in the tail partitions on
# HW (observed; matches the "<128-partition matmuls problematic" note).
RPC = 64

PC_RANGE = (-59.9, -59.9, -2.0, 59.9, 59.9, 5.9)
TEMPERATURE = 0.1
LOSS_SCALE = 0.01
SQRT_INV_TEMP = float(np.sqrt(1.0 / TEMPERATURE))

F32 = mybir.dt.float32
I32 = mybir.dt.int32

# Observability for test.py: exec_time_ns of the last run of each pass
# (populated only when KERNEL_TRACE=1 so profiling is on).
LAST_EXEC_NS = {"gather": None, "loss": None}
LAST_TRACE_DIRS = {"gather": None, "loss": None}
_DEBUG = {}


def _install_ntff_hook():
    """Provide antenv.axon_hooks (absent in this image) so bass_utils'
    trace=True path can capture NTFF profiles via the axon PJRT .so."""
    try:
        import antenv.axon_hooks  # noqa: F401
        return
    except ImportError:
        pass
    hook = None
    so_path = "/opt/axon/libaxon_pjrt.so"
    if os.path.exists(so_path):
        lib = ctypes.CDLL(so_path)
        if hasattr(lib, "axon_start_nrt_profile"):
            lib.axon_start_nrt_profile.argtypes = [
                ctypes.POINTER(ctypes.c_int64), ctypes.c_size_t]
            lib.axon_start_nrt_profile.restype = ctypes.c_int64
            lib.axon_stop_nrt_profile.argtypes = [ctypes.c_char_p]
            lib.axon_stop_nrt_profile.restype = ctypes.c_int64

            @contextlib.contextmanager
            def _hook(output_dir, device_ids):
                import jax
                jax.devices()
                if device_ids:
                    ids = (ctypes.c_int64 * len(device_ids))(*device_ids)
                    rc = lib.axon_start_nrt_profile(ids, len(device_ids))
                else:
                    rc = lib.axon_start_nrt_profile(None, 0)
                if rc != 0:
                    raise RuntimeError(f"axon_start_nrt_profile rc={rc}")
                try:
                    yield
                finally:
                    n = lib.axon_stop_nrt_profile(str(output_dir).encode())
                    print(f"profile: {n} file(s) -> {output_dir}", file=sys.stderr)

            hook = _hook
    mod = types.ModuleType("antenv.axon_hooks")
    mod.get_axon_ntff_profile_hook = lambda: hook
    mod.set_axon_ntff_profile_hook = lambda h: None
    sys.modules["antenv.axon_hooks"] = mod


def _run(prog_key, in_maps):
    """run_bass_kernel_spmd with env-gated tracing."""
    progs = _progs()
    if os.environ.get("KERNEL_TRACE"):
        _install_ntff_hook()
        # Artifact upload needs network egress; keep everything local.
        _bass_utils.upload_artifacts = lambda tmpdir: "local://" + str(tmpdir)
        import tempfile
        tmpdir = tempfile.mkdtemp(prefix=f"bass_{prog_key}_")
        LAST_TRACE_DIRS[prog_key] = tmpdir
        res = run_bass_kernel_spmd(
            progs[prog_key], in_maps, core_ids=list(range(NCORES)),
            trace=True, tmpdir=tmpdir,
        )
    else:
        # Never let a stray BASS_TRACE in the environment route us into the
        # trace path (its antenv import may be unavailable).
        old = os.environ.get("BASS_NEVER_TRACE")
        os.environ["BASS_NEVER_TRACE"] = "1"
        try:
            res = run_bass_kernel_spmd(
                progs[prog_key], in_maps, core_ids=list(range(NCORES)))
        finally:
            if old is None:
                os.environ.pop("BASS_NEVER_TRACE", None)
            else:
                os.environ["BASS_NEVER_TRACE"] = old
    LAST_EXEC_NS[prog_key] = res.exec_time_ns
    return res


NPAD = 512          # pixels padded to 4 full 128-partition tiles
NTILES = NPAD // 128


def _build_gather_prog():
    # Raw bass (no TileContext): the trimmed walrus pipeline here can't
    # codegen Tile's tail drain, and raw bass also skips the exit barrier.
    #
    # HW indirect-DMA semantics (verified on device): each index addresses a
    # CONTIGUOUS run of rowsize elements at table_flat[idx*coef], one index
    # per dest partition; source AP strides are not honored.  So the table is
    # uploaded pixel-major [HW, CPC] and each gathered "row" is one pixel's
    # CPC-channel slice (64B contiguous).  4 instructions gather 512 padded
    # pixel rows into [128, NTILES*CPC], pixel-major.
    nc = bass.Bass("TRN2", target_bir_lowering=False)
    table = nc.dram_tensor("table", [HW, CPC], F32, kind="ExternalInput")
    idx = nc.dram_tensor("idx", [128, NTILES], I32, kind="ExternalInput")
    feats_out = nc.dram_tensor("feats", [128, NTILES * CPC], F32,
                               kind="ExternalOutput")
    with (
        nc.sbuf_tensor([128, NTILES], I32) as idx_sb,
        nc.sbuf_tensor([128, NTILES * CPC], F32) as feats_sb,
        nc.semaphore() as sem,
        nc.Block() as block,
    ):
        @block.gpsimd
        def _(g):
            g.dma_start(idx_sb[:], idx[:]).then_inc(sem, 16)
            g.wait_ge(sem, 16)
            for t in range(NTILES):
                g.indirect_dma_start(
                    out=feats_sb[:, t * CPC:(t + 1) * CPC],
                    out_offset=None,
                    in_=table[:],
                    in_offset=bass.IndirectOffsetOnAxis(
                        ap=idx_sb[:, t:t + 1], axis=0),
                ).then_inc(sem, 16)
            g.wait_ge(sem, 16 + 16 * NTILES)
            g.dma_start(feats_out[:], feats_sb[:]).then_inc(sem, 16)
            g.wait_ge(sem, 32 + 16 * NTILES)
    return nc


# Packed single-input layout for the loss pass: columns [0,N) = feats,
# [N, N+RPC) = this core's lhs block, column N+RPC = labels (f32 values,
# rows 0..RPC-1), column N+RPC+1 = row-valid mask.
PACKW = N + RPC + 2


def _build_loss_prog():
    # Raw bass (Tile's tail drain can't be codegen'd by this walrus build).
    # One shared semaphore serializes the cross-engine chain; every wait
    # carries exactly one condition.
    nc = bass.Bass("TRN2", target_bir_lowering=False)
    packed = nc.dram_tensor("packed", [C, PACKW], F32, kind="ExternalInput")
    out = nc.dram_tensor("partial", [1, 1], F32, kind="ExternalOutput")
    valout = nc.dram_tensor("valout", [C, 1], F32, kind="ExternalOutput")

    AF = mybir.ActivationFunctionType
    OP = mybir.AluOpType

    from contextlib import ExitStack
    with ExitStack() as ctx:
        packed_sb = ctx.enter_context(nc.sbuf_tensor([C, PACKW], F32))
        ones_cc = ctx.enter_context(nc.sbuf_tensor([C, C], F32))
        sq = ctx.enter_context(nc.sbuf_tensor([C, N], F32))
        sql = ctx.enter_context(nc.sbuf_tensor([C, RPC], F32))
        inv = ctx.enter_context(nc.sbuf_tensor([C, N], F32))
        invr = ctx.enter_context(nc.sbuf_tensor([C, RPC], F32))
        featsN = ctx.enter_context(nc.sbuf_tensor([C, N], F32))
        lhsN = ctx.enter_context(nc.sbuf_tensor([C, RPC], F32))
        iota_j = ctx.enter_context(nc.sbuf_tensor([RPC, N], F32))
        lmask = ctx.enter_context(nc.sbuf_tensor([RPC, N], F32))
        escr = ctx.enter_context(nc.sbuf_tensor([RPC, N], F32))
        mscr = ctx.enter_context(nc.sbuf_tensor([RPC, N], F32))
        rowmax = ctx.enter_context(nc.sbuf_tensor([RPC, 1], F32))
        negmax = ctx.enter_context(nc.sbuf_tensor([RPC, 1], F32))
        ssum = ctx.enter_context(nc.sbuf_tensor([RPC, 1], F32))
        logS = ctx.enter_context(nc.sbuf_tensor([RPC, 1], F32))
        pick = ctx.enter_context(nc.sbuf_tensor([RPC, 1], F32))
        val = ctx.enter_context(nc.sbuf_tensor([C, 1], F32))
        ones_r = ctx.enter_context(nc.sbuf_tensor([C, 1], F32))
        res = ctx.enter_context(nc.sbuf_tensor([1, 1], F32))
        css = ctx.enter_context(nc.psum_tensor([C, N], F32))
        rss = ctx.enter_context(nc.psum_tensor([C, RPC], F32))
        G = ctx.enter_context(nc.psum_tensor([RPC, N], F32))
        tot = ctx.enter_context(nc.psum_tensor([1, 1], F32))
        s = ctx.enter_context(nc.semaphore())
        block = ctx.enter_context(nc.Block())
        feats_sb = packed_sb[:, 0:N]
        lhs_sb = packed_sb[:, N:N + RPC]
        lab_sb = packed_sb[0:RPC, N + RPC:N + RPC + 1]
        # full-height row mask: rows >= RPC are 0, zeroing pad partitions of
        # val regardless of their prior contents
        mask_sb = packed_sb[0:C, N + RPC + 1:N + RPC + 2]

        # Raw-bass hazard discipline: compute-engine sem updates can fire
        # before the data write lands (observed on HW for TensorReduce and
        # cross-engine consumers), so every release that another engine waits
        # on rides on a drain() of the producing engine.

        @block.gpsimd
        def _(g):
            g.iota(iota_j[:], pattern=[[1, N]], base=0, channel_multiplier=0,
                   allow_small_or_imprecise_dtypes=True)
            g.drain().then_inc(s, 1)                                      # 1
            g.dma_start(packed_sb[:], packed[:]).then_inc(s, 16)          # 17
            g.wait_ge(s, 44)
            g.dma_start(out[:], res[:]).then_inc(s, 16)                   # 60
            g.dma_start(valout[:], val[:]).then_inc(s, 16)                # 76
            g.wait_ge(s, 76)

        @block.vector
        def _(v):
            v.memset(ones_cc[:], 1.0).then_inc(s, 1)                      # 18*
            v.memset(ones_r[:], 1.0).then_inc(s, 1)
            v.memset(val[:], 0.0).then_inc(s, 1)                          # 20*
            v.wait_ge(s, 20)      # iota-drain + packed DMA + own memsets
            v.tensor_mul(sq[:], feats_sb, feats_sb).then_inc(s, 1)        # 21
            v.tensor_mul(sql[:], lhs_sb, lhs_sb).then_inc(s, 1)           # 22
            v.tensor_scalar(out=lmask[:], in0=iota_j[:], scalar1=lab_sb,
                            scalar2=None, op0=OP.is_equal)
            v.drain().then_inc(s, 1)                                      # 23 -> PE
            v.wait_ge(s, 27)      # inv/invr sqrt'd and drained by ACT
            # drains between every dependent same-engine step: a consumer can
            # otherwise read the tail of the producer's output before it lands
            v.tensor_scalar_max(out=inv[:], in0=inv[:], scalar1=1e-12)
            v.drain().then_inc(s, 1)                                      # 28
            v.reciprocal(out=inv[:], in_=inv[:])
            v.drain().then_inc(s, 1)                                      # 29
            v.scalar_tensor_tensor(out=featsN[:], in0=inv[:], scalar=SQRT_INV_TEMP,
                                   in1=feats_sb, op0=OP.mult, op1=OP.mult)
            v.drain().then_inc(s, 1)                                      # 30
            v.tensor_scalar_max(out=invr[:], in0=invr[:], scalar1=1e-12)
            v.drain().then_inc(s, 1)                                      # 31
            v.reciprocal(out=invr[:], in_=invr[:])
            v.drain().then_inc(s, 1)                                      # 32
            v.scalar_tensor_tensor(out=lhsN[:], in0=invr[:], scalar=SQRT_INV_TEMP,
                                   in1=lhs_sb, op0=OP.mult, op1=OP.mult)
            v.drain().then_inc(s, 1)                                      # 33 -> PE
            v.wait_ge(s, 34)      # G in PSUM (drained)
            v.reduce_max(out=negmax[:], in_=G[:], axis=mybir.AxisListType.X,
                         negate=True)
            v.drain().then_inc(s, 1)                                      # 35 -> ACT (negmax)
            v.tensor_mul(mscr[:], G[:], lmask[:])
            v.drain().then_inc(s, 1)                                      # 36
            v.reduce_sum(out=pick[:], in_=mscr[:], axis=mybir.AxisListType.X)
            v.drain().then_inc(s, 1)                                      # 37 (pick landed)
            v.wait_ge(s, 39)      # ssum + logS landed (ACT drains)
            v.tensor_add(val[0:RPC, :], pick[:], negmax[:])
            v.drain().then_inc(s, 1)                                      # 40
            v.tensor_sub(val[0:RPC, :], val[0:RPC, :], logS[:])
            v.drain().then_inc(s, 1)                                      # 41
            v.tensor_mul(val[:], val[:], mask_sb)
            v.drain().then_inc(s, 1)                                      # 42 -> PE

        @block.tensor
        def _(t):
            t.wait_ge(s, 23)      # sq, sql, ones ready
            nc.tensor.matmul(css[:], lhsT=ones_cc[:], rhs=sq[:],
                             start=True, stop=True).then_inc(s, 1)        # 24
            nc.tensor.matmul(rss[:], lhsT=ones_cc[:], rhs=sql[:],
                             start=True, stop=True)
            t.drain().then_inc(s, 1)                                      # 25 -> ACT
            t.wait_ge(s, 33)      # featsN, lhsN ready
            nc.tensor.matmul(G[:], lhsT=lhsN[:], rhs=featsN[:],
                             start=True, stop=True)
            t.drain().then_inc(s, 1)                                      # 34 -> DVE/ACT
            t.wait_ge(s, 42)      # val ready
            nc.tensor.matmul(tot[:], lhsT=val[:], rhs=ones_r[:],
                             start=True, stop=True)
            t.drain().then_inc(s, 1)                                      # 43 -> ACT

        @block.scalar
        def _(a):
            a.wait_ge(s, 25)      # css, rss in PSUM
            a.activation(out=inv[:], in_=css[:], func=AF.Sqrt).then_inc(s, 1)   # 26
            a.activation(out=invr[:], in_=rss[:], func=AF.Sqrt)
            a.drain().then_inc(s, 1)                                      # 27 -> DVE
            a.wait_ge(s, 35)      # negmax landed (G via 34)
            a.activation(out=escr[:], in_=G[:], func=AF.Exp, bias=negmax[:],
                         scale=1.0, accum_out=ssum[:])
            a.drain().then_inc(s, 1)                                      # 38 (ssum landed)
            a.activation(out=logS[:], in_=ssum[:], func=AF.Ln)
            a.drain().then_inc(s, 1)                                      # 39 -> DVE
            a.wait_ge(s, 43)      # tot in PSUM
            a.mul(res[:], tot[:], 1.0)
            a.drain().then_inc(s, 1)                                      # 44 -> Pool DMA
    return nc


_PROGS = {}


def _progs():
    if not _PROGS:
        _PROGS["gather"] = _build_gather_prog()
        _PROGS["loss"] = _build_loss_prog()
    return _PROGS


def _pixel_indices(gt_boxes: np.ndarray) -> np.ndarray:
    """Exact fp32 replication of the reference pixel-index math (last batch)."""
    boxes = np.asarray(gt_boxes)[B - 1].astype(np.float32, copy=False)
    x = boxes[:, 0].astype(np.float32)
    y = boxes[:, 1].astype(np.float32)
    span_w = PC_RANGE[3] - PC_RANGE[0]
    span_h = PC_RANGE[4] - PC_RANGE[1]
    cx = (x - np.float32(PC_RANGE[0])) / np.float32(span_w) * np.float32(W)
    cy = (y - np.float32(PC_RANGE[1])) / np.float32(span_h) * np.float32(H)
    cx = np.clip(cx.astype(np.int32), 0, W - 1)
    cy = np.clip(cy.astype(np.int32), 0, H - 1)
    return (cy.astype(np.int64) * W + cx.astype(np.int64)).astype(np.int32)


def kernel(spatial_features_2d, gt_boxes, static_labels, dynamic_labels,
           num_static=None, **_unused):
    progs = _progs()
    sf = np.asarray(spatial_features_2d)
    pix = _pixel_indices(gt_boxes)  # [N] int32, linear index into one H*W plane

    # ---- Pass 1: channel-sharded row gather -------------------------------
    # Each core's table shard is uploaded pixel-major [HW, CPC] so a pixel's
    # channel slice is one contiguous 64B row; the index tile is [128, NTILES]
    # (pixel t*128+p at idx[p, t]), matching one index per dest partition.
    pix_pad = np.zeros(NPAD, dtype=np.int32)
    pix_pad[:N] = pix
    idx = np.ascontiguousarray(pix_pad.reshape(NTILES, 128).T)  # [128, NTILES]
    sf3 = sf[B - 1].reshape(C, HW)
    in_maps = [
        {
            "table": np.ascontiguousarray(
                sf3[k * CPC:(k + 1) * CPC].T, dtype=np.float32),  # [HW, CPC]
            "idx": idx,
        }
        for k in range(NCORES)
    ]
    r1 = _run("gather", in_maps)
    # results[k]["feats"]: [128, NTILES*CPC], feats[p, t*CPC+c] =
    # table[pix[t*128+p], c] -> reassemble to [C, N].
    parts = []
    for k in range(NCORES):
        a = r1.results[k]["feats"].reshape(128, NTILES, CPC)
        a = a.transpose(2, 1, 0).reshape(CPC, NPAD)  # [CPC, NTILES*128]
        parts.append(a[:, :N])
    feats = np.ascontiguousarray(np.concatenate(parts, axis=0))  # [C, N]
    _DEBUG["feats"] = feats
    _DEBUG["pix"] = pix

    # ---- Pass 2: row-sharded contrastive loss -----------------------------
    labels = np.concatenate(
        [np.asarray(static_labels), np.asarray(dynamic_labels)], axis=0
    ).astype(np.int64)
    lab_pad = np.zeros(NCORES * RPC, dtype=np.float32)
    lab_pad[:N] = labels.astype(np.float32)
    msk_pad = np.zeros(NCORES * RPC, dtype=np.float32)
    msk_pad[:N] = 1.0
    lhs_pad = np.zeros((C, NCORES * RPC), dtype=np.float32)
    lhs_pad[:, :N] = feats

    in_maps = []
    for k in range(NCORES):
        packed = np.zeros((C, PACKW), dtype=np.float32)
        packed[:, :N] = feats
        packed[:, N:N + RPC] = lhs_pad[:, k * RPC:(k + 1) * RPC]
        packed[:RPC, N + RPC] = lab_pad[k * RPC:(k + 1) * RPC]
        packed[:RPC, N + RPC + 1] = msk_pad[k * RPC:(k + 1) * RPC]
        # rows RPC..C-1 of the mask column stay 0 -> they zero val's pad rows
        in_maps.append({"packed": packed})
    r2 = _run("loss", in_maps)
    _DEBUG["partials"] = [float(r2.results[k]["partial"][0, 0]) for k in range(NCORES)]
    _DEBUG["vals"] = [r2.results[k]["valout"][:, 0].copy() for k in range(NCORES)]
    total = float(sum(float(r2.results[k]["partial"][0, 0]) for k in range(NCORES)))
    loss = np.float32(total * (-LOSS_SCALE / N))
    return np.array(loss, dtype=np.float32)



# revision 13
# speedup vs baseline: 1.7294x; 1.7294x over previous
"""Trainium2 kernel for DetContrastiveLoss (embedding_lookup).

Reference semantics (buggy original preserved): only the LAST batch element of
spatial_features_2d / gt_boxes is used.  500 box centers are mapped to pixel
indices, the 128-channel feature vector at each pixel is gathered from the
128 MB feature map resident in device HBM, L2-normalized, and a 500x500
cosine-similarity contrastive loss (log_softmax + label pick) is reduced to a
scalar.

Single-launch, single-core design (v2 of this problem; v1 used two launches
and measured 66 us -- most of it was a second NEFF preamble, a host
round-trip between passes, and a [128,1] column DMA that cost 6.4 us in
descriptor spray):

  - The full pixel-major table [H*W, C] lives in HBM.  gpsimd issues 4
    indirect DMAs (128 indices each, one 512 B channel row per pixel) into a
    pixel-major SBUF tile [128, 4*128].
  - Per tile, a 5-stage pipeline across engines: DVE tensor_tensor_reduce
    gives per-pixel sum-of-squares [128,1]; ACT computes 1/norm as
    exp(-0.5*ln(css)) (Ln and Exp share one activation table --
    natural_log_exp_and_others -- so after a dummy-exp prefetch there are no
    table loads on the critical path; Rsqrt is banned in bass); DVE scales
    the raw tile by 1/norm (per-partition scalar); PE transposes the
    normalized tile (identity comes in as an input DMA); ACT copies PSUM ->
    SBUF.
  - PE computes the Gram matrix as 4 matmuls [128, 500] (lhsT = 128-column
    block of the normalized channel-major features).  ACT exponentiates each
    block with scale=1/T and accumulates row sums (softmax denominator)
    for free; DVE picks the labeled logit per row with a fused
    mask-multiply-reduce against a HOST-precomputed one-hot label mask.
  - val = pick - ln(ssum) rows are dotted with a validity mask and reduced
    across partitions by a [1,1] matmul; ACT copies the scalar out of PSUM
    and issues the 4-byte output DMA itself (avoiding a cross-engine hop and
    the column-DMA descriptor spray).

Host does: pixel index math (exact fp32 replication), one-hot label mask,
table transpose to pixel-major, and the final -LOSS_SCALE/N scaling.
"""

import contextlib
import ctypes
import os
import sys
import types

import numpy as np

from concourse import bass, mybir
from concourse import bass_utils as _bass_utils
from concourse.bass_utils import run_bass_kernel_spmd

# Problem geometry (hardcoded per spec nn_DetContrastiveLoss_72636486910298).
B, C, H, W = 4, 128, 512, 512
HW = H * W
N = 500
NT = 4                  # pixel tiles of 128 (500 padded to 512)
NPAD = NT * 128

PC_RANGE = (-59.9, -59.9, -2.0, 59.9, 59.9, 5.9)
TEMPERATURE = 0.1
LOSS_SCALE = 0.01
INV_T = float(1.0 / TEMPERATURE)

F32 = mybir.dt.float32
I32 = mybir.dt.int32

# Observability for test.py: exec_time_ns of the last run (populated only when
# KERNEL_TRACE=1 so profiling is on).
LAST_EXEC_NS = {"main": None}
LAST_TRACE_DIRS = {"main": None}
_DEBUG = {}


def _install_ntff_hook():
    """Provide antenv.axon_hooks (absent in this image) so bass_utils'
    trace=True path can capture NTFF profiles via the axon PJRT .so."""
    try:
        import antenv.axon_hooks  # noqa: F401
        return
    except ImportError:
        pass
    hook = None
    so_path = "/opt/axon/libaxon_pjrt.so"
    if os.path.exists(so_path):
        lib = ctypes.CDLL(so_path)
        if hasattr(lib, "axon_start_nrt_profile"):
            lib.axon_start_nrt_profile.argtypes = [
                ctypes.POINTER(ctypes.c_int64), ctypes.c_size_t]
            lib.axon_start_nrt_profile.restype = ctypes.c_int64
            lib.axon_stop_nrt_profile.argtypes = [ctypes.c_char_p]
            lib.axon_stop_nrt_profile.restype = ctypes.c_int64

            @contextlib.contextmanager
            def _hook(output_dir, device_ids):
                import jax
                jax.devices()
                if device_ids:
                    ids = (ctypes.c_int64 * len(device_ids))(*device_ids)
                    rc = lib.axon_start_nrt_profile(ids, len(device_ids))
                else:
                    rc = lib.axon_start_nrt_profile(None, 0)
                if rc != 0:
                    raise RuntimeError(f"axon_start_nrt_profile rc={rc}")
                try:
                    yield
                finally:
                    n = lib.axon_stop_nrt_profile(str(output_dir).encode())
                    print(f"profile: {n} file(s) -> {output_dir}", file=sys.stderr)

            hook = _hook
    mod = types.ModuleType("antenv.axon_hooks")
    mod.get_axon_ntff_profile_hook = lambda: hook
    mod.set_axon_ntff_profile_hook = lambda h: None
    sys.modules["antenv.axon_hooks"] = mod


def _run(prog_key, in_maps, core_ids):
    """run_bass_kernel_spmd with env-gated tracing."""
    progs = _progs()
    if os.environ.get("KERNEL_TRACE"):
        _install_ntff_hook()
        # Artifact upload needs network egress; keep everything local.
        _bass_utils.upload_artifacts = lambda tmpdir: "local://" + str(tmpdir)
        import tempfile
        tmpdir = tempfile.mkdtemp(prefix=f"bass_{prog_key}_")
        LAST_TRACE_DIRS[prog_key] = tmpdir
        res = run_bass_kernel_spmd(
            progs[prog_key], in_maps, core_ids=core_ids,
            trace=True, tmpdir=tmpdir,
        )
    else:
        # Never let a stray BASS_TRACE in the environment route us into the
        # trace path (its antenv import may be unavailable).
        old = os.environ.get("BASS_NEVER_TRACE")
        os.environ["BASS_NEVER_TRACE"] = "1"
        try:
            res = run_bass_kernel_spmd(progs[prog_key], in_maps,
                                       core_ids=core_ids)
        finally:
            if old is None:
                os.environ.pop("BASS_NEVER_TRACE", None)
            else:
                os.environ["BASS_NEVER_TRACE"] = old
    LAST_EXEC_NS[prog_key] = res.exec_time_ns
    return res


def _build_main_prog(stage="full"):
    # stage: debug knob -- "front" stops after the rhsN copies and DMAs
    # rhsN out; "gram" adds G0 matmuls + ssum exps and DMAs sspk out;
    # "full" is the real kernel.
    # Raw bass (no TileContext): the trimmed walrus pipeline here can't
    # codegen Tile's tail drain, and raw bass also skips the exit barrier.
    #
    # HW indirect-DMA semantics (verified on device in the v1 session): each
    # index addresses a CONTIGUOUS run of rowsize elements at
    # table_flat[idx*coef], one index per dest partition; source AP strides
    # are not honored.  Table is pixel-major [HW, C]; one index gathers one
    # pixel's full 512 B channel row.
    nc = bass.Bass("TRN2", target_bir_lowering=False)
    table = nc.dram_tensor("table", [HW, C], F32, kind="ExternalInput")
    idx_d = nc.dram_tensor("idx", [128, NT], I32, kind="ExternalInput")
    ident_d = nc.dram_tensor("ident", [128, 128], F32, kind="ExternalInput")
    lmask_d = nc.dram_tensor("lmask", [128, NT * N], F32, kind="ExternalInput")
    rmask_d = nc.dram_tensor("rmask", [128, NT], F32, kind="ExternalInput")
    out_d = nc.dram_tensor("partial", [1, 1], F32, kind="ExternalOutput")
    dbg_rhs_d = (nc.dram_tensor("dbg_rhs", [128, NPAD], F32, kind="ExternalOutput")
                 if stage == "front" else None)
    dbg_sspk_d = (nc.dram_tensor("dbg_sspk", [128, 2 * NT], F32,
                                 kind="ExternalOutput")
                  if stage == "gram" else None)

    AF = mybir.ActivationFunctionType
    OP = mybir.AluOpType
    F32R = mybir.dt.float32r

    from contextlib import ExitStack
    with ExitStack() as ctx:
        def sb(name, shape, dt=F32):
            return ctx.enter_context(nc.sbuf_tensor(name, shape, dt))
        idx_sb = sb("idx_sb", [128, NT], I32)
        I_sb = sb("I_sb", [128, 128])
        lmask_sb = sb("lmask_sb", [128, NT * N])
        rmask_sb = sb("rmask_sb", [128, NT])
        traw = sb("traw", [128, NPAD])    # gathered pixel-major raw features
        trawN = sb("trawN", [128, NPAD])  # per-pixel normalized features
        # float32r: PE runs 1 cycle/row (vs 4 for fp32); the ACT copies
        # below perform the required fp32r rounding on write.
        rhsN = sb("rhsN", [128, NPAD], F32R)  # channel-major normalized
        sqscr = sb("sqscr", [128, 128])   # ACT square scratch (unused)
        css = sb("css", [128, NT])        # per-pixel sum of squares
        lncss = sb("lncss", [128, NT])
        inv = sb("inv", [128, NT])        # 1/norm per pixel
        escrs = [sb(f"escr{m}", [128, N]) for m in range(NT)]  # exp(sim/T)
        mscrs = [sb(f"mscr{m}", [128, N]) for m in range(NT)]  # pick products
        sspk = sb("sspk", [128, 2 * NT])  # cols 0:NT softmax denominators,
                                          # cols NT:2NT picked exp(logit)
        lnall = sb("lnall", [128, 2 * NT])
        val = sb("val", [128, NT])
        vscr = sb("vscr", [128, NT])
        tmp = sb("tmp", [128, 1])
        ones_r = sb("ones_r", [128, 1])
        scr0 = sb("scr0", [1, 1])
        scr1 = sb("scr1", [1, 1])
        res = sb("res", [1, 1])
        # HW quirk (bisected): 4 transpose matmuls into one full PSUM bank
        # fault the NEFF; 2x2 across two half-banks is fine.
        Thats = [ctx.enter_context(nc.psum_tensor(f"That{i}", [128, 256], F32))
                 for i in range(2)]
        G0 = [ctx.enter_context(nc.psum_tensor(f"G0_{m}", [128, N], F32))
              for m in range(NT)]
        tot = ctx.enter_context(nc.psum_tensor("tot", [1, 1], F32))
        # One semaphore per DMA milestone: transfers sharing a queue
        # complete out-of-order across the 16 SDMA engines, so a shared
        # counter's intermediate values are not sound sync points (CoreSim
        # SemaphoreRace caught this).
        sidx = ctx.enter_context(nc.semaphore())
        sid = ctx.enter_context(nc.semaphore())
        smsk = ctx.enter_context(nc.semaphore())
        sg = [ctx.enter_context(nc.semaphore(name=f"sg{t}"))
              for t in range(NT)]
        smul = ctx.enter_context(nc.semaphore())
        vsem = ctx.enter_context(nc.semaphore())
        asem = ctx.enter_context(nc.semaphore())
        psem = ctx.enter_context(nc.semaphore())
        osem = ctx.enter_context(nc.semaphore())
        block = ctx.enter_context(nc.Block())

        # Raw-bass hazard discipline (v1 lesson): compute-engine sem updates
        # can fire before the data write lands, so every release that another
        # engine (or a dependent same-engine op) consumes rides on a drain()
        # of the producing engine.  DMA completion increments are safe as-is.

        @block.gpsimd
        def _(g):
            # All DMAs ride gpsimd's qPoolDynamic (the SW-DGE path the
            # two-pass version validated on this runtime); the input issues
            # overlap the idx DMA's completion latency.
            g.dma_start(idx_sb[:], idx_d[:]).then_inc(sidx, 16)
            g.dma_start(I_sb[:], ident_d[:]).then_inc(sid, 16)
            g.dma_start(lmask_sb[:], lmask_d[:]).then_inc(smsk, 16)
            g.dma_start(rmask_sb[:], rmask_d[:]).then_inc(smsk, 16)
            g.wait_ge(sidx, 16)
            for t in range(NT):
                g.indirect_dma_start(
                    out=traw[:, t * 128:(t + 1) * 128],
                    out_offset=None,
                    in_=table[:],
                    in_offset=bass.IndirectOffsetOnAxis(
                        ap=idx_sb[:, t:t + 1], axis=0),
                ).then_inc(sg[t], 16)
            if stage == "front":
                g.wait_ge(asem, 9)
                g.dma_start(dbg_rhs_d[:], rhsN[:].bitcast(F32)).then_inc(osem, 16)
                g.wait_ge(osem, 16)
                return
            if stage == "gram":
                g.wait_ge(asem, 13)
                g.dma_start(dbg_sspk_d[:], sspk[:]).then_inc(osem, 16)
                g.wait_ge(osem, 16)
                return
            # Pick products: gpsimd is otherwise idle after the gathers; DVE
            # does the (free-axis) reductions it alone supports.
            g.wait_ge(smsk, 32)     # label mask (and rmask) landed
            for m in range(NT):
                g.wait_ge(asem, 10 + m)
                g.tensor_mul(mscrs[m][:], escrs[m][:],
                             lmask_sb[:, m * N:(m + 1) * N])
                g.drain().then_inc(smul, 1)                          # smul m+1
            g.wait_ge(asem, 15)     # res holds the scalar
            g.dma_start(out_d[:], res[:]).then_inc(osem, 16)
            g.wait_ge(osem, 16)

        @block.vector
        def _(v):
            v.memset(scr0[:], 0.0)
            v.memset(ones_r[:], 1.0)
            v.drain().then_inc(vsem, 1)                              # v1
            for t in range(NT):
                tile = slice(t * 128, (t + 1) * 128)
                v.wait_ge(asem, 2 + t)
                v.tensor_scalar_mul(
                    out=trawN[:, tile], in0=traw[:, tile],
                    scalar1=inv[:, t:t + 1])
                v.drain().then_inc(vsem, 1)                          # v2+t
            if stage != "full":
                return
            for m in range(NT):
                v.wait_ge(smul, m + 1)
                v.reduce_sum(out=sspk[:, NT + m:NT + m + 1],
                             in_=mscrs[m][:], axis=mybir.AxisListType.X)
                v.drain().then_inc(vsem, 1)                          # v6+m
            v.wait_ge(asem, 14)     # lnall ready
            # val = ln(exp(logit_pick)) - ln(sum exp) per row
            v.tensor_sub(val[:], lnall[:, NT:2 * NT], lnall[:, 0:NT])
            v.drain()
            v.wait_ge(smsk, 32)     # row-validity mask landed
            v.tensor_mul(vscr[:], val[:], rmask_sb[:])
            v.drain()
            v.reduce_sum(out=tmp[:], in_=vscr[:], axis=mybir.AxisListType.X)
            v.drain().then_inc(vsem, 1)                              # v10

        @block.scalar
        def _(a):
            a.wait_ge(vsem, 1)
            # Dummy exp: triggers the (single) activation table load during
            # the idx-DMA latency window instead of on the critical path.
            a.activation(out=scr1[:], in_=scr0[:], func=AF.Exp)
            a.drain().then_inc(asem, 1)                              # a1
            for t in range(NT):
                tile = slice(t * 128, (t + 1) * 128)
                a.wait_ge(sg[t], 16)
                # sum-of-squares per pixel comes free from the Square
                # activation's row accumulator
                a.activation(out=sqscr[:], in_=traw[:, tile], func=AF.Square,
                             accum_out=css[:, t:t + 1])
                a.drain()
                a.activation(out=lncss[:, t:t + 1], in_=css[:, t:t + 1],
                             func=AF.Ln)
                a.drain()
                # 1/norm = exp(-0.5 * ln(css))
                a.activation(out=inv[:, t:t + 1], in_=lncss[:, t:t + 1],
                             func=AF.Exp, scale=-0.5)
                a.drain().then_inc(asem, 1)                          # a2+t
            for t in range(NT):
                tile = slice(t * 128, (t + 1) * 128)
                a.wait_ge(psem, 1 + t)
                a.copy(out=rhsN[:, tile],
                       in_=Thats[t // 2][:, (t % 2) * 128:(t % 2 + 1) * 128])
                a.drain().then_inc(asem, 1)                          # a6+t
            if stage == "front":
                return
            for m in range(NT):
                a.wait_ge(psem, 5 + m)
                a.activation(out=escrs[m][:], in_=G0[m][:, 0:N], func=AF.Exp,
                             scale=INV_T, accum_out=sspk[:, m:m + 1])
                a.drain().then_inc(asem, 1)                          # a10+m
            if stage == "gram":
                return
            a.wait_ge(vsem, 9)      # picked-exp reductions landed
            a.activation(out=lnall[:], in_=sspk[:], func=AF.Ln)
            a.drain().then_inc(asem, 1)                              # a14
            a.wait_ge(psem, 9)
            a.mul(res[:], tot[:], 1.0)
            a.drain().then_inc(asem, 1)                              # a15

        @block.tensor
        def _(te):
            te.wait_ge(sid, 16)     # identity landed
            for t in range(NT):
                tile = slice(t * 128, (t + 1) * 128)
                te.wait_ge(vsem, 2 + t)
                nc.tensor.transpose(
                    Thats[t // 2][:, (t % 2) * 128:(t % 2 + 1) * 128],
                    trawN[:, tile], I_sb[:])
                te.drain().then_inc(psem, 1)                         # p1+t
            if stage == "front":
                return
            te.wait_ge(asem, 9)     # all PSUM->SBUF copies done
            for m in range(NT):
                # float32r runs the PE at 1 cycle/row (vs 4 for fp32); the
                # tolerance (2e-2) dwarfs the precision delta.
                nc.tensor.matmul(
                    G0[m][:, 0:N], lhsT=rhsN[:, m * 128:(m + 1) * 128],
                    rhs=rhsN[:, 0:N], start=True, stop=True)
                te.drain().then_inc(psem, 1)                         # p5+m
            if stage == "gram":
                return
            te.wait_ge(vsem, 10)
            nc.tensor.matmul(tot[:], lhsT=tmp[:], rhs=ones_r[:],
                             start=True, stop=True)
            te.drain().then_inc(psem, 1)                             # p9
    return nc


_PROGS = {}


def _progs():
    if not _PROGS:
        _PROGS["main"] = _build_main_prog()
    return _PROGS


def _pixel_indices(gt_boxes: np.ndarray) -> np.ndarray:
    """Exact fp32 replication of the reference pixel-index math (last batch)."""
    boxes = np.asarray(gt_boxes)[B - 1].astype(np.float32, copy=False)
    x = boxes[:, 0].astype(np.float32)
    y = boxes[:, 1].astype(np.float32)
    span_w = PC_RANGE[3] - PC_RANGE[0]
    span_h = PC_RANGE[4] - PC_RANGE[1]
    cx = (x - np.float32(PC_RANGE[0])) / np.float32(span_w) * np.float32(W)
    cy = (y - np.float32(PC_RANGE[1])) / np.float32(span_h) * np.float32(H)
    cx = np.clip(cx.astype(np.int32), 0, W - 1)
    cy = np.clip(cy.astype(np.int32), 0, H - 1)
    return (cy.astype(np.int64) * W + cx.astype(np.int64)).astype(np.int32)


def kernel(spatial_features_2d, gt_boxes, static_labels, dynamic_labels,
           num_static=None, **_unused):
    sf = np.asarray(spatial_features_2d)
    pix = _pixel_indices(gt_boxes)  # [N] int32, linear index into H*W plane

    # Pixel-major table: one 512 B contiguous channel row per pixel.
    table = np.ascontiguousarray(
        sf[B - 1].reshape(C, HW).T, dtype=np.float32)   # [HW, C]

    pix_pad = np.zeros(NPAD, dtype=np.int32)
    pix_pad[:N] = pix
    idx = np.ascontiguousarray(pix_pad.reshape(NT, 128).T)  # [128, NT]

    labels = np.concatenate(
        [np.asarray(static_labels), np.asarray(dynamic_labels)], axis=0
    ).astype(np.int64)
    # One-hot label mask, row-block-major: block m in columns [m*N,(m+1)*N).
    lmask = np.zeros((128, NT * N), dtype=np.float32)
    rmask = np.zeros((128, NT), dtype=np.float32)
    for r in range(NT * 128):
        m, p = divmod(r, 128)
        if r < N:
            lmask[p, m * N + int(labels[r])] = 1.0
            rmask[p, m] = 1.0
        else:
            # invalid rows still need a finite ln(picked-exp); rmask zeroes
            # their contribution, but 0*ln(0) would be NaN
            lmask[p, m * N] = 1.0

    ident = np.eye(128, dtype=np.float32)

    in_maps = [{
        "table": table,
        "idx": idx,
        "ident": ident,
        "lmask": lmask,
        "rmask": rmask,
    }]
    r = _run("main", in_maps, core_ids=[0])
    total = float(r.results[0]["partial"][0, 0])
    _DEBUG["total"] = total
    loss = np.float32(total * (-LOSS_SCALE / N))
    return np.array(loss, dtype=np.float32)


# revision 19
# speedup vs baseline: 1.7339x; 1.0026x over previous
"""Trainium2 kernel for DetContrastiveLoss (embedding_lookup).

Reference semantics (buggy original preserved): only the LAST batch element of
spatial_features_2d / gt_boxes is used.  500 box centers are mapped to pixel
indices, the 128-channel feature vector at each pixel is gathered from the
128 MB feature map resident in device HBM, L2-normalized, and a 500x500
cosine-similarity contrastive loss (log_softmax + label pick) is reduced to a
scalar.

Single-launch, single-core design (v2 of this problem; v1 used two launches
and measured 66 us -- most of it was a second NEFF preamble, a host
round-trip between passes, and a [128,1] column DMA that cost 6.4 us in
descriptor spray):

  - The full pixel-major table [H*W, C] lives in HBM.  gpsimd issues 4
    indirect DMAs (128 indices each, one 512 B channel row per pixel) into a
    pixel-major SBUF tile [128, 4*128].
  - Per tile, a 5-stage pipeline across engines: DVE tensor_tensor_reduce
    gives per-pixel sum-of-squares [128,1]; ACT computes 1/norm as
    exp(-0.5*ln(css)) (Ln and Exp share one activation table --
    natural_log_exp_and_others -- so after a dummy-exp prefetch there are no
    table loads on the critical path; Rsqrt is banned in bass); DVE scales
    the raw tile by 1/norm (per-partition scalar); PE transposes the
    normalized tile (identity comes in as an input DMA); ACT copies PSUM ->
    SBUF.
  - PE computes the Gram matrix as 4 matmuls [128, 500] (lhsT = 128-column
    block of the normalized channel-major features).  ACT exponentiates each
    block with scale=1/T and accumulates row sums (softmax denominator)
    for free; DVE picks the labeled logit per row with a fused
    mask-multiply-reduce against a HOST-precomputed one-hot label mask.
  - val = pick - ln(ssum) rows are dotted with a validity mask and reduced
    across partitions by a [1,1] matmul; ACT copies the scalar out of PSUM
    and issues the 4-byte output DMA itself (avoiding a cross-engine hop and
    the column-DMA descriptor spray).

Host does: pixel index math (exact fp32 replication), one-hot label mask,
table transpose to pixel-major, and the final -LOSS_SCALE/N scaling.
"""

import contextlib
import ctypes
import os
import sys
import types

import numpy as np

from concourse import bass, mybir
from concourse import bass_utils as _bass_utils
from concourse.bass_utils import run_bass_kernel_spmd

# Problem geometry (hardcoded per spec nn_DetContrastiveLoss_72636486910298).
B, C, H, W = 4, 128, 512, 512
HW = H * W
N = 500
NT = 4                  # pixel tiles of 128 (500 padded to 512)
NPAD = NT * 128

PC_RANGE = (-59.9, -59.9, -2.0, 59.9, 59.9, 5.9)
TEMPERATURE = 0.1
LOSS_SCALE = 0.01
INV_T = float(1.0 / TEMPERATURE)

F32 = mybir.dt.float32
I32 = mybir.dt.int32

# Observability for test.py: exec_time_ns of the last run (populated only when
# KERNEL_TRACE=1 so profiling is on).
LAST_EXEC_NS = {"main": None}
LAST_TRACE_DIRS = {"main": None}
_DEBUG = {}


def _install_ntff_hook():
    """Provide antenv.axon_hooks (absent in this image) so bass_utils'
    trace=True path can capture NTFF profiles via the axon PJRT .so."""
    try:
        import antenv.axon_hooks  # noqa: F401
        return
    except ImportError:
        pass
    hook = None
    so_path = "/opt/axon/libaxon_pjrt.so"
    if os.path.exists(so_path):
        lib = ctypes.CDLL(so_path)
        if hasattr(lib, "axon_start_nrt_profile"):
            lib.axon_start_nrt_profile.argtypes = [
                ctypes.POINTER(ctypes.c_int64), ctypes.c_size_t]
            lib.axon_start_nrt_profile.restype = ctypes.c_int64
            lib.axon_stop_nrt_profile.argtypes = [ctypes.c_char_p]
            lib.axon_stop_nrt_profile.restype = ctypes.c_int64

            @contextlib.contextmanager
            def _hook(output_dir, device_ids):
                import jax
                jax.devices()
                if device_ids:
                    ids = (ctypes.c_int64 * len(device_ids))(*device_ids)
                    rc = lib.axon_start_nrt_profile(ids, len(device_ids))
                else:
                    rc = lib.axon_start_nrt_profile(None, 0)
                if rc != 0:
                    raise RuntimeError(f"axon_start_nrt_profile rc={rc}")
                try:
                    yield
                finally:
                    n = lib.axon_stop_nrt_profile(str(output_dir).encode())
                    print(f"profile: {n} file(s) -> {output_dir}", file=sys.stderr)

            hook = _hook
    mod = types.ModuleType("antenv.axon_hooks")
    mod.get_axon_ntff_profile_hook = lambda: hook
    mod.set_axon_ntff_profile_hook = lambda h: None
    sys.modules["antenv.axon_hooks"] = mod


def _run(prog_key, in_maps, core_ids):
    """run_bass_kernel_spmd with env-gated tracing."""
    progs = _progs()
    if os.environ.get("KERNEL_TRACE"):
        _install_ntff_hook()
        # Artifact upload needs network egress; keep everything local.
        _bass_utils.upload_artifacts = lambda tmpdir: "local://" + str(tmpdir)
        import tempfile
        tmpdir = tempfile.mkdtemp(prefix=f"bass_{prog_key}_")
        LAST_TRACE_DIRS[prog_key] = tmpdir
        res = run_bass_kernel_spmd(
            progs[prog_key], in_maps, core_ids=core_ids,
            trace=True, tmpdir=tmpdir,
        )
    else:
        # Never let a stray BASS_TRACE in the environment route us into the
        # trace path (its antenv import may be unavailable).
        old = os.environ.get("BASS_NEVER_TRACE")
        os.environ["BASS_NEVER_TRACE"] = "1"
        try:
            res = run_bass_kernel_spmd(progs[prog_key], in_maps,
                                       core_ids=core_ids)
        finally:
            if old is None:
                os.environ.pop("BASS_NEVER_TRACE", None)
            else:
                os.environ["BASS_NEVER_TRACE"] = old
    LAST_EXEC_NS[prog_key] = res.exec_time_ns
    return res


def _build_main_prog(stage="full"):
    # stage: debug knob -- "front" stops after the rhsN copies and DMAs
    # rhsN out; "gram" adds G0 matmuls + ssum exps and DMAs sspk out;
    # "full" is the real kernel.
    # Raw bass (no TileContext): the trimmed walrus pipeline here can't
    # codegen Tile's tail drain, and raw bass also skips the exit barrier.
    #
    # HW indirect-DMA semantics (verified on device in the v1 session): each
    # index addresses a CONTIGUOUS run of rowsize elements at
    # table_flat[idx*coef], one index per dest partition; source AP strides
    # are not honored.  Table is pixel-major [HW, C]; one index gathers one
    # pixel's full 512 B channel row.
    nc = bass.Bass("TRN2", target_bir_lowering=False)
    table = nc.dram_tensor("table", [HW, C], F32, kind="ExternalInput")
    idx_d = nc.dram_tensor("idx", [128, NT], I32, kind="ExternalInput")
    ident_d = nc.dram_tensor("ident", [128, 128], F32, kind="ExternalInput")
    lmask_d = nc.dram_tensor("lmask", [128, NT * N], F32, kind="ExternalInput")
    rmask_d = nc.dram_tensor("rmask", [128, NT], F32, kind="ExternalInput")
    out_d = nc.dram_tensor("partial", [1, 1], F32, kind="ExternalOutput")
    dbg_rhs_d = (nc.dram_tensor("dbg_rhs", [128, NPAD], F32, kind="ExternalOutput")
                 if stage == "front" else None)
    dbg_ssum_d = (nc.dram_tensor("dbg_ssum", [128, NT], F32,
                                 kind="ExternalOutput")
                  if stage == "gram" else None)

    AF = mybir.ActivationFunctionType
    OP = mybir.AluOpType
    F32R = mybir.dt.float32r

    from contextlib import ExitStack
    with ExitStack() as ctx:
        def sb(name, shape, dt=F32):
            return ctx.enter_context(nc.sbuf_tensor(name, shape, dt))
        idx_sb = sb("idx_sb", [128, NT], I32)
        I_sb = sb("I_sb", [128, 128])
        lmask_sb = sb("lmask_sb", [128, NT * N])
        rmask_sb = sb("rmask_sb", [128, NT])
        traw = sb("traw", [128, NPAD])    # gathered pixel-major raw features
        trawN = sb("trawN", [128, NPAD])  # per-pixel normalized features
        # float32r: PE runs 1 cycle/row (vs 4 for fp32); the ACT copies
        # below perform the required fp32r rounding on write.
        rhsN = sb("rhsN", [128, NPAD], F32R)  # channel-major normalized
        sqscr = sb("sqscr", [128, 128])   # ACT square scratch (unused)
        css = sb("css", [128, NT])        # per-pixel sum of squares
        lncss = sb("lncss", [128, NT])
        inv = sb("inv", [128, NT])        # 1/norm per pixel
        escrs = [sb(f"escr{m}", [128, N]) for m in range(NT)]  # exp(sim/T)
        mscrs = [sb(f"mscr{m}", [128, N]) for m in range(NT)]  # pick products
        sspk = sb("sspk", [128, 2 * NT])  # cols 0:NT softmax denominators,
                                          # cols NT:2NT picked exp(logit)
        lnall = sb("lnall", [128, 2 * NT])
        val = sb("val", [128, NT])
        vscr = sb("vscr", [128, NT])
        tmp = sb("tmp", [128, 1])
        ones_r = sb("ones_r", [128, 1])
        scr0 = sb("scr0", [1, 1])
        scr1 = sb("scr1", [1, 1])
        res = sb("res", [1, 1])
        # HW quirk (bisected): 4 transpose matmuls into one full PSUM bank
        # fault the NEFF; 2x2 across two half-banks is fine.
        Thats = [ctx.enter_context(nc.psum_tensor(f"That{i}", [128, 256], F32))
                 for i in range(2)]
        G0 = [ctx.enter_context(nc.psum_tensor(f"G0_{m}", [128, N], F32))
              for m in range(NT)]
        tot = ctx.enter_context(nc.psum_tensor("tot", [1, 1], F32))
        # One semaphore per DMA milestone: transfers sharing a queue
        # complete out-of-order across the 16 SDMA engines, so a shared
        # counter's intermediate values are not sound sync points (CoreSim
        # SemaphoreRace caught this).
        sidx = ctx.enter_context(nc.semaphore())
        sid = ctx.enter_context(nc.semaphore())
        smsk = ctx.enter_context(nc.semaphore())
        sg = [ctx.enter_context(nc.semaphore(name=f"sg{t}"))
              for t in range(NT)]
        smul = ctx.enter_context(nc.semaphore(name="smul"))
        vsem = ctx.enter_context(nc.semaphore())
        asem = ctx.enter_context(nc.semaphore())
        psem = ctx.enter_context(nc.semaphore())
        osem = ctx.enter_context(nc.semaphore())
        block = ctx.enter_context(nc.Block())

        # Raw-bass hazard discipline (v1 lesson): compute-engine sem updates
        # can fire before the data write lands, so every release that another
        # engine (or a dependent same-engine op) consumes rides on a drain()
        # of the producing engine.  DMA completion increments are safe as-is.

        @block.gpsimd
        def _(g):
            # All DMAs on gpsimd's qPoolDynamic: the SP/ACT HW-DGE rings
            # error out on this runtime (bisected on HW).
            g.dma_start(idx_sb[:], idx_d[:]).then_inc(sidx, 16)
            g.dma_start(I_sb[:], ident_d[:]).then_inc(sid, 16)
            g.dma_start(lmask_sb[:], lmask_d[:]).then_inc(smsk, 16)
            g.dma_start(rmask_sb[:], rmask_d[:]).then_inc(smsk, 16)
            g.wait_ge(sidx, 16)
            for t in range(NT):
                g.indirect_dma_start(
                    out=traw[:, t * 128:(t + 1) * 128],
                    out_offset=None,
                    in_=table[:],
                    in_offset=bass.IndirectOffsetOnAxis(
                        ap=idx_sb[:, t:t + 1], axis=0),
                ).then_inc(sg[t], 16)
            if stage == "full":
                # Pick products off the exp outputs (SBUF); gpsimd cannot
                # read PSUM.  DVE does the free-axis reductions.
                g.wait_ge(smsk, 32)
                for m in range(NT):
                    g.wait_ge(asem, 10 + m)
                    g.tensor_mul(mscrs[m][:], escrs[m][:],
                                 lmask_sb[:, m * N:(m + 1) * N])
                    g.drain().then_inc(smul, 1)                      # smul m+1
                g.wait_ge(asem, 15)     # res holds the scalar
                g.dma_start(out_d[:], res[:]).then_inc(osem, 16)
                g.wait_ge(osem, 16)
            if stage == "front":
                g.wait_ge(asem, 9)
                g.dma_start(dbg_rhs_d[:], rhsN[:].bitcast(F32)).then_inc(osem, 16)
                g.wait_ge(osem, 16)
                return

        @block.vector
        def _(v):
            v.memset(scr0[:], 0.0)
            v.memset(ones_r[:], 1.0)
            v.drain().then_inc(vsem, 1)                              # v1
            for t in range(NT):
                tile = slice(t * 128, (t + 1) * 128)
                v.wait_ge(asem, 2 + t)
                v.tensor_scalar_mul(
                    out=trawN[:, tile], in0=traw[:, tile],
                    scalar1=inv[:, t:t + 1])
                v.drain().then_inc(vsem, 1)                          # v2+t
            if stage != "full":
                return
            for m in range(NT):
                v.wait_ge(smul, m + 1)
                v.reduce_sum(out=sspk[:, NT + m:NT + m + 1],
                             in_=mscrs[m][:], axis=mybir.AxisListType.X)
                v.drain().then_inc(vsem, 1)                          # v6+m
            v.wait_ge(asem, 14)     # lnall ready
            # val = ln(exp(logit_pick)) - ln(sum exp) per row
            v.tensor_sub(val[:], lnall[:, NT:2 * NT], lnall[:, 0:NT])
            v.drain()
            v.wait_ge(smsk, 32)     # row-validity mask landed
            v.tensor_mul(vscr[:], val[:], rmask_sb[:])
            v.drain()
            v.reduce_sum(out=tmp[:], in_=vscr[:], axis=mybir.AxisListType.X)
            v.drain().then_inc(vsem, 1)                              # v10

        @block.scalar
        def _(a):
            a.wait_ge(vsem, 1)
            # Dummy exp: triggers the (single) activation table load during
            # the idx-DMA latency window instead of on the critical path.
            a.activation(out=scr1[:], in_=scr0[:], func=AF.Exp)
            a.drain().then_inc(asem, 1)                              # a1
            for t in range(NT):
                tile = slice(t * 128, (t + 1) * 128)
                a.wait_ge(sg[t], 16)
                # sum-of-squares per pixel comes free from the Square
                # activation's row accumulator
                a.activation(out=sqscr[:], in_=traw[:, tile], func=AF.Square,
                             accum_out=css[:, t:t + 1])
                a.drain()
                a.activation(out=lncss[:, t:t + 1], in_=css[:, t:t + 1],
                             func=AF.Ln)
                a.drain()
                # 1/norm = exp(-0.5 * ln(css))
                a.activation(out=inv[:, t:t + 1], in_=lncss[:, t:t + 1],
                             func=AF.Exp, scale=-0.5)
                a.drain().then_inc(asem, 1)                          # a2+t
            for t in range(NT):
                tile = slice(t * 128, (t + 1) * 128)
                a.wait_ge(psem, 1 + t)
                a.copy(out=rhsN[:, tile],
                       in_=Thats[t // 2][:, (t % 2) * 128:(t % 2 + 1) * 128])
                a.drain().then_inc(asem, 1)                          # a6+t
            if stage == "front":
                return
            for m in range(NT):
                a.wait_ge(psem, 5 + m)
                a.activation(out=escrs[m][:], in_=G0[m][:, 0:N], func=AF.Exp,
                             scale=INV_T, accum_out=sspk[:, m:m + 1])
                a.drain().then_inc(asem, 1)                          # a10+m
            if stage == "gram":
                a.dma_start(dbg_ssum_d[:], sspk[:, 0:NT]).then_inc(osem, 16)
                a.wait_ge(osem, 16)
                return
            a.wait_ge(vsem, 9)      # picked-exp reductions landed
            a.activation(out=lnall[:], in_=sspk[:], func=AF.Ln)
            a.drain().then_inc(asem, 1)                              # a14
            a.wait_ge(psem, 9)
            a.mul(res[:], tot[:], 1.0)
            a.drain().then_inc(asem, 1)                              # a15

        @block.tensor
        def _(te):
            te.wait_ge(sid, 16)     # identity landed
            for t in range(NT):
                tile = slice(t * 128, (t + 1) * 128)
                te.wait_ge(vsem, 2 + t)
                nc.tensor.transpose(
                    Thats[t // 2][:, (t % 2) * 128:(t % 2 + 1) * 128],
                    trawN[:, tile], I_sb[:])
                te.drain().then_inc(psem, 1)                         # p1+t
            if stage == "front":
                return
            te.wait_ge(asem, 9)     # all PSUM->SBUF copies done
            for m in range(NT):
                # float32r runs the PE at 1 cycle/row (vs 4 for fp32); the
                # tolerance (2e-2) dwarfs the precision delta.
                nc.tensor.matmul(
                    G0[m][:, 0:N], lhsT=rhsN[:, m * 128:(m + 1) * 128],
                    rhs=rhsN[:, 0:N], start=True, stop=True)
                te.drain().then_inc(psem, 1)                         # p5+m
            if stage == "gram":
                return
            te.wait_ge(vsem, 10)
            nc.tensor.matmul(tot[:], lhsT=tmp[:], rhs=ones_r[:],
                             start=True, stop=True)
            te.drain().then_inc(psem, 1)                             # p9
    return nc


_PROGS = {}


def _progs():
    if not _PROGS:
        _PROGS["main"] = _build_main_prog()
    return _PROGS


def _pixel_indices(gt_boxes: np.ndarray) -> np.ndarray:
    """Exact fp32 replication of the reference pixel-index math (last batch)."""
    boxes = np.asarray(gt_boxes)[B - 1].astype(np.float32, copy=False)
    x = boxes[:, 0].astype(np.float32)
    y = boxes[:, 1].astype(np.float32)
    span_w = PC_RANGE[3] - PC_RANGE[0]
    span_h = PC_RANGE[4] - PC_RANGE[1]
    cx = (x - np.float32(PC_RANGE[0])) / np.float32(span_w) * np.float32(W)
    cy = (y - np.float32(PC_RANGE[1])) / np.float32(span_h) * np.float32(H)
    cx = np.clip(cx.astype(np.int32), 0, W - 1)
    cy = np.clip(cy.astype(np.int32), 0, H - 1)
    return (cy.astype(np.int64) * W + cx.astype(np.int64)).astype(np.int32)


def kernel(spatial_features_2d, gt_boxes, static_labels, dynamic_labels,
           num_static=None, **_unused):
    sf = np.asarray(spatial_features_2d)
    pix = _pixel_indices(gt_boxes)  # [N] int32, linear index into H*W plane

    # Pixel-major table: one 512 B contiguous channel row per pixel.
    table = np.ascontiguousarray(
        sf[B - 1].reshape(C, HW).T, dtype=np.float32)   # [HW, C]

    pix_pad = np.zeros(NPAD, dtype=np.int32)
    pix_pad[:N] = pix
    idx = np.ascontiguousarray(pix_pad.reshape(NT, 128).T)  # [128, NT]

    labels = np.concatenate(
        [np.asarray(static_labels), np.asarray(dynamic_labels)], axis=0
    ).astype(np.int64)
    # One-hot label mask, row-block-major: block m in columns [m*N,(m+1)*N).
    lmask = np.zeros((128, NT * N), dtype=np.float32)
    rmask = np.zeros((128, NT), dtype=np.float32)
    for r in range(NT * 128):
        m, p = divmod(r, 128)
        if r < N:
            lmask[p, m * N + int(labels[r])] = 1.0
            rmask[p, m] = 1.0
        else:
            # invalid rows still need a finite ln(picked-exp); rmask zeroes
            # their contribution, but 0*ln(0) would be NaN
            lmask[p, m * N] = 1.0

    ident = np.eye(128, dtype=np.float32)

    in_maps = [{
        "table": table,
        "idx": idx,
        "ident": ident,
        "lmask": lmask,
        "rmask": rmask,
    }]
    r = _run("main", in_maps, core_ids=[0])
    total = float(r.results[0]["partial"][0, 0])
    _DEBUG["total"] = total
    loss = np.float32(total * (-LOSS_SCALE / N))
    return np.array(loss, dtype=np.float32)
